# revision 1
# baseline (speedup 1.0000x reference)
"""LSTM encoder kernel for Trainium2 (Bass/Tile), data-parallel over batch.

Problem: single-layer LSTM, B=64, T=2048, D=64, H=128, PyTorch gate order
(i, f, g, o).  Each of the 8 cores runs the full sequential scan over its
8-row batch shard; weights are replicated.

Layout ("gates on partitions"): per step the gate pre-activations live in
PSUM as (128 partitions = hidden unit, free = 4 gate slots x 8 batch).
The x-projection for a 16-step chunk is computed by 4 wide matmuls into a
PSUM bank (one bank = 16 steps x 32 cols) and the recurrent W_hh @ h^T
matmuls accumulate on top (start=False).  Activations read PSUM directly;
the cell/hidden updates are small (128, 8) DVE ops.  h is staged in an
SBUF (128, 128) tile per chunk (col = b*16 + t), PE-transposed at chunk
end to (b,t) partitions, and DMA'd straight from PSUM to the output.
"""

import numpy as np

import concourse.bass as bass
import concourse.mybir as mybir
import concourse.tile as tile
from concourse import bacc
from concourse.bass_utils import run_bass_kernel_spmd
from concourse.masks import make_identity

# Problem constants (hardcoded per harness contract).
B, T, D, H = 64, 2048, 64, 128
N_CORES = 8
RB = B // N_CORES           # batch rows per core
CHUNK = 16                  # steps per PSUM bank (16 * 32 fp32 cols = 2KB)
N_CHUNKS = T // CHUNK
F32 = mybir.dt.float32

# Gate slots in the per-step PSUM slice, ordered so sigmoid gates (i, f, o)
# are contiguous in cols 0:24 and tanh gate (g) is cols 24:32.
# Value = row-block index into the (4H, ...) weights, PyTorch order i,f,g,o.
SLOTS = [0, 1, 3, 2]        # slot k -> weight block; slots = [i, f, o, g]


def build_lstm_bass(t_steps: int = T) -> bass.Bass:
    n_chunks = t_steps // CHUNK
    nc = bacc.Bacc("TRN2", target_bir_lowering=False)

    x = nc.dram_tensor("input_data", [RB, T, D], F32, kind="ExternalInput")
    w_ih = nc.dram_tensor("W_ih", [4 * H, D], F32, kind="ExternalInput")
    w_hh = nc.dram_tensor("W_hh", [4 * H, H], F32, kind="ExternalInput")
    b_ih = nc.dram_tensor("b_ih", [4 * H], F32, kind="ExternalInput")
    b_hh = nc.dram_tensor("b_hh", [4 * H], F32, kind="ExternalInput")
    h0 = nc.dram_tensor("h0", [RB, H], F32, kind="ExternalInput")
    c0 = nc.dram_tensor("c0", [RB, H], F32, kind="ExternalInput")
    out = nc.dram_tensor("out", [RB, T, H], F32, kind="ExternalOutput")

    SIG = mybir.ActivationFunctionType.Sigmoid
    TANH = mybir.ActivationFunctionType.Tanh

    with tile.TileContext(nc) as tc:
        with (
            tc.tile_pool(name="const", bufs=1) as const,
            tc.tile_pool(name="wload", bufs=2) as wload,
            tc.tile_pool(name="xnat", bufs=3) as xnat_p,
            tc.tile_pool(name="xT", bufs=3) as xT_p,
            tc.tile_pool(name="acts", bufs=4) as acts_p,
            tc.tile_pool(name="small", bufs=4) as small_p,
            tc.tile_pool(name="hstage", bufs=3) as hstage_p,
            tc.tile_pool(name="pbank", bufs=2, space="PSUM") as pbank_p,
            tc.tile_pool(name="tpsum", bufs=2, space="PSUM") as tpsum_p,
            tc.tile_pool(name="hpsum", bufs=2, space="PSUM") as hpsum_p,
        ):
            identity = const.tile([128, 128], F32, tag="ident")
            make_identity(nc, identity)

            # ---- weights: W_hh blocks transposed to lhsT (K=H, M=128) ----
            whh_T = []
            for k, blk in enumerate(SLOTS):
                wnat = wload.tile([128, H], F32, tag="wnat")
                nc.sync.dma_start(wnat[:], w_hh[blk * 128 : (blk + 1) * 128, :])
                ps = tpsum_p.tile([H, 128], F32, tag="tps")
                nc.tensor.transpose(ps[:], wnat[:], identity[:])
                wt = const.tile([H, 128], F32, tag=f"whh{k}")
                nc.vector.tensor_copy(wt[:], ps[:])
                whh_T.append(wt)

            # ---- W_ih blocks transposed + bias row (K=D+1, M=128) ----
            bsum = const.tile([1, 4 * H], F32, tag="bsum")
            btmp = wload.tile([1, 4 * H], F32, tag="btmp")
            nc.sync.dma_start(bsum[:], b_ih.rearrange("(a n) -> a n", a=1))
            nc.sync.dma_start(btmp[:], b_hh.rearrange("(a n) -> a n", a=1))
            nc.vector.tensor_add(bsum[:], bsum[:], btmp[:])

            wih_T = []
            for k, blk in enumerate(SLOTS):
                wnat = wload.tile([128, D], F32, tag="wnat")
                nc.sync.dma_start(wnat[:], w_ih[blk * 128 : (blk + 1) * 128, :])
                ps = tpsum_p.tile([D, 128], F32, tag="tps")
                nc.tensor.transpose(ps[:], wnat[:], identity[:])
                wt = const.tile([D + 1, 128], F32, tag=f"wih{k}")
                nc.vector.tensor_copy(wt[0:D, :], ps[:])
                # bias row lives on partition D; cross-partition move via DMA
                nc.sync.dma_start(
                    wt[D : D + 1, :], bsum[0:1, blk * 128 : (blk + 1) * 128]
                )
                wih_T.append(wt)

            # ---- initial state h0/c0 -> (H, RB) ----
            snat = wload.tile([RB, H], F32, tag="snat")
            nc.sync.dma_start(snat[:], h0[:, :])
            ps = tpsum_p.tile([H, RB], F32, tag="tps")
            nc.tensor.transpose(ps[:], snat[:], identity[0:RB, 0:RB])
            hT0 = const.tile([H, RB], F32, tag="hT0")
            nc.vector.tensor_copy(hT0[:], ps[:])

            snat = wload.tile([RB, H], F32, tag="snat")
            nc.sync.dma_start(snat[:], c0[:, :])
            ps = tpsum_p.tile([H, RB], F32, tag="tps")
            nc.tensor.transpose(ps[:], snat[:], identity[0:RB, 0:RB])
            cT = const.tile([H, RB], F32, tag="cT")
            nc.vector.tensor_copy(cT[:], ps[:])

            # ---- main scan ----
            h_prev = hT0[:, :]  # AP of the rhs for the next step's matmuls
            for c in range(n_chunks):
                t0 = c * CHUNK

                # x chunk: (RB,16,D) -> (128,(b t)) -> transpose -> (D+1,128)
                xt_nat = xnat_p.tile([RB * CHUNK, D], F32, tag="xnat")
                nc.sync.dma_start(xt_nat[:], x[:, t0 : t0 + CHUNK, :])
                xps = tpsum_p.tile([D, RB * CHUNK], F32, tag="tps")
                nc.tensor.transpose(xps[:], xt_nat[:], identity[:])
                xT = xT_p.tile([D + 1, RB * CHUNK], F32, tag="xT")
                nc.vector.tensor_copy(xT[0:D, :], xps[:])
                nc.gpsimd.memset(xT[D : D + 1, :], 1.0)

                # x-projection prefill: 4 matmuls, N = 128 (b outer, t inner)
                pb = pbank_p.tile([128, CHUNK * 32], F32, tag="pb")
                pb_btg = pb.rearrange("p (t g b) -> p b t g", t=CHUNK, g=4, b=RB)
                for k in range(4):
                    nc.tensor.matmul(
                        pb_btg[:, :, :, k],
                        wih_T[k][:],
                        xT[:],
                        start=(k == 0),
                        stop=False,
                        skip_group_check=True,
                    )

                pb_step = pb.rearrange("p (t x) -> p t x", t=CHUNK)
                hstage = hstage_p.tile([128, RB * CHUNK], F32, tag="hstage")
                hs_bt = hstage.rearrange("p (b t) -> p b t", b=RB)

                for s in range(CHUNK):
                    # recurrent matmuls accumulate onto the x-projection
                    for k in range(4):
                        nc.tensor.matmul(
                            pb_step[:, s, k * RB : (k + 1) * RB],
                            whh_T[k][:],
                            h_prev,
                            start=False,
                            stop=True,
                            skip_group_check=True,
                        )

                    acts = acts_p.tile([128, 4 * RB], F32, tag="acts")
                    nc.scalar.activation(
                        acts[:, 0 : 3 * RB], pb_step[:, s, 0 : 3 * RB], SIG
                    )
                    nc.scalar.activation(
                        acts[:, 3 * RB : 4 * RB], pb_step[:, s, 3 * RB : 4 * RB], TANH
                    )

                    ig = small_p.tile([H, RB], F32, tag="ig")
                    fc = small_p.tile([H, RB], F32, tag="fc")
                    nc.vector.tensor_mul(ig[:], acts[:, 0:RB], acts[:, 3 * RB : 4 * RB])
                    nc.vector.tensor_mul(fc[:], acts[:, RB : 2 * RB], cT[:])
                    nc.vector.tensor_add(cT[:], ig[:], fc[:])

                    tanc = small_p.tile([H, RB], F32, tag="tanc")
                    nc.scalar.activation(tanc[:], cT[:], TANH)

                    h_col = hs_bt[:, :, s]
                    nc.vector.tensor_mul(h_col, acts[:, 2 * RB : 3 * RB], tanc[:])
                    h_prev = h_col

                # transpose h chunk to (b,t) partitions and store
                hps = hpsum_p.tile([RB * CHUNK, H], F32, tag="hps")
                nc.tensor.transpose(hps[:], hstage[:], identity[:])
                ostage = hstage_p.tile([RB * CHUNK, H], F32, tag="ostage")
                nc.vector.tensor_copy(ostage[:], hps[:])
                nc.sync.dma_start(out[:, t0 : t0 + CHUNK, :], ostage[:])

    nc.compile()
    return nc


_NC_CACHE: dict[int, bass.Bass] = {}


def kernel(
    input_data: np.ndarray,
    W_ih: np.ndarray,
    W_hh: np.ndarray,
    b_ih: np.ndarray,
    b_hh: np.ndarray,
    h0: np.ndarray,
    c0: np.ndarray,
    _t_steps: int = T,
    _trace: bool = False,
):
    nc = _NC_CACHE.get(_t_steps)
    if nc is None:
        nc = build_lstm_bass(_t_steps)
        _NC_CACHE[_t_steps] = nc

    reps = {
        "W_ih": np.ascontiguousarray(W_ih, np.float32),
        "W_hh": np.ascontiguousarray(W_hh, np.float32),
        "b_ih": np.ascontiguousarray(b_ih, np.float32),
        "b_hh": np.ascontiguousarray(b_hh, np.float32),
    }
    in_maps = []
    for k in range(N_CORES):
        sl = slice(k * RB, (k + 1) * RB)
        m = dict(reps)
        m["input_data"] = np.ascontiguousarray(input_data[sl], np.float32)
        m["h0"] = np.ascontiguousarray(h0[sl], np.float32)
        m["c0"] = np.ascontiguousarray(c0[sl], np.float32)
        in_maps.append(m)

    res = run_bass_kernel_spmd(
        nc, in_maps, core_ids=list(range(N_CORES)), trace=_trace
    )
    full = np.concatenate([r["out"] for r in res.results], axis=0)
    if _trace:
        return full, res
    return full

